# revision 13
# baseline (speedup 1.0000x reference)
"""Trainium2 Bass kernel for nn_Attention_Temp_1468878815458.

Math: the reference computes
    pos   = arange(S) @ Wp.T + bp                       # (S,)
    embed = x.squeeze(1) + pos[:, None]                 # (B,S,D)
    v/k/q = embed @ {Wv,Wk,Wq}.T
    scores[b,x,y]  = (sum_q queries[b,q,x]) * (sum_k keys[b,k,y])
    attention      = softmax(scores, axis=1)            # over x
    out[b,v,y]     = sum_x attention[b,x,y] * sum_n values[b,v,n]

Since softmax normalizes over axis=1 and is then *summed* over axis=1,
sum_x attention[b,x,y] == 1 exactly.  Therefore
    out[b,s,y] = sum_n values[b,s,n]
               = (x[b,0,s,:] + pos[s]) . wv      for every y,
where wv[d] = sum_n Wv[n,d].  The kernel streams x once, computes the
per-row weighted sum with wv, adds the per-s bias pos[s]*sum(wv), and
broadcasts the scalar across the last dim.

Sharding: pure data parallel over batch, 1024 batches per core.  Each
core's shard is viewed as (128 partitions, 6144 f32): partition p holds
64 consecutive rows (8 batches x 8 seq) contiguously -> fully
contiguous DMA in AND out (24KB runs/partition).
"""

import numpy as np

import concourse.bass as bass
import concourse.mybir as mybir
from concourse.bass import broadcast_tensor_aps
from concourse.bass_utils import run_bass_kernel_spmd
from concourse.tile import TileContext

N_CORES = 8
B, S, D = 8192, 8, 96
BPC = B // N_CORES          # 1024 batches per core
ROWS = BPC * S              # 8192 rows of length D per core
P = 128                     # SBUF partitions
FREE = ROWS * D // P        # 6144 f32 per partition
RPP = ROWS // P             # 64 rows per partition
NCH = 4                     # pipeline chunks (4 in + 4 out = 8 HWDGE DMAs
                            # = exactly the 8 DMAHW sem lanes; no lane reuse
                            # -> every DMA carries at most 1 sync wait)
CHR = RPP // NCH            # rows per partition per chunk
CHF = CHR * D               # f32 per partition per chunk

_NC_CACHE = None


def _build() -> bass.Bass:
    # seq codegen lowers multi-wait sync (e.g. the kernel-tail drain) to
    # sequencer commands; this walrus build allows only 1 wait per inst
    nc = bass.Bass(use_seq_codegen=True)
    x = nc.declare_dram_parameter("x", [P, FREE], mybir.dt.float32, isOutput=False)
    # combined constants: [:, :D] = wv replicated, [:, D:D+RPP] = per-row bias
    wb = nc.declare_dram_parameter("wb", [P, D + RPP], mybir.dt.float32, isOutput=False)
    out = nc.declare_dram_parameter("out", [P, FREE], mybir.dt.float32, isOutput=True)

    with TileContext(nc) as tc:
        with (
            tc.tile_pool(name="const", bufs=1) as cpool,
            # one buffer per chunk: no slot reuse -> no WAR waits on DMAs
            # (walrus allows at most 1 sync wait per DMA/TT instruction)
            tc.tile_pool(name="xp", bufs=NCH) as xpool,
            tc.tile_pool(name="pp", bufs=2) as ppool,
            tc.tile_pool(name="op", bufs=NCH) as opool,
            tc.tile_pool(name="rp", bufs=NCH) as rpool,
        ):
            wb_sb = cpool.tile([P, D + RPP], mybir.dt.float32)
            # SWDGE: keeps the 8 HWDGE sem lanes free for the x/out streams
            nc.gpsimd.dma_start(out=wb_sb[:], in_=wb[:])
            wv_sb = wb_sb[:, :D]
            bias_sb = wb_sb[:, D : D + RPP]
            # priming op: absorbs the wb DMA wait so per-chunk TTs carry
            # only their own x-chunk DMA wait (HW limit: 1 wait per TT)
            prime = rpool.tile([P, 1], mybir.dt.float32, tag="prime")
            nc.vector.reduce_sum(
                out=prime[:], in_=wb_sb[:], axis=mybir.AxisListType.X
            )

            for c in range(NCH):
                xt = xpool.tile([P, CHF], mybir.dt.float32, tag="xt")
                nc.sync.dma_start(out=xt[:], in_=x[:, c * CHF : (c + 1) * CHF])

                x3 = xt[:].rearrange("p (r d) -> p r d", d=D)
                wv3 = wv_sb.rearrange("p (r d) -> p r d", r=1)
                _, wv3b = broadcast_tensor_aps(x3, wv3)
                pt = ppool.tile([P, CHF], mybir.dt.float32, tag="pt")
                p3 = pt[:].rearrange("p (r d) -> p r d", d=D)
                nc.vector.tensor_tensor(
                    out=p3, in0=x3, in1=wv3b, op=mybir.AluOpType.mult
                )

                rd = rpool.tile([P, CHR], mybir.dt.float32, tag="rd")
                nc.vector.reduce_sum(out=rd[:], in_=p3, axis=mybir.AxisListType.X)
                nc.vector.tensor_add(
                    out=rd[:], in0=rd[:], in1=bias_sb[:, c * CHR : (c + 1) * CHR]
                )

                ot = opool.tile([P, CHF], mybir.dt.float32, tag="ot")
                ot3 = ot[:].rearrange("p (r d) -> p r d", d=D)
                rd3 = rd[:].rearrange("p (r d) -> p r d", d=1)
                _, rd3b = broadcast_tensor_aps(ot3, rd3)
                nc.scalar.copy(out=ot3, in_=rd3b)

                nc.sync.dma_start(out=out[:, c * CHF : (c + 1) * CHF], in_=ot[:])
    _split_multi_waits(nc)
    return nc


def _split_multi_waits(nc: bass.Bass) -> None:
    """Walrus (this build) allows only one sync wait per instruction.

    Tile's kernel-tail drain merges waits on every DMA lane + engine sem
    into one instruction; split the extras onto same-engine NOPs placed
    immediately before it.
    """
    for f in nc.m.functions:
        for bb in f.blocks:
            insts = bb.instructions
            i = 0
            while i < len(insts):
                inst = insts[i]
                si = inst.sync_info
                if si is not None and si.on_wait and len(si.on_wait) > 1:
                    waits = list(si.on_wait)
                    nops = []
                    for j, w in enumerate(waits[:-1]):
                        nop = mybir.InstNoOp(
                            name=f"{inst.name}-wsplit{j}", ins=[], outs=[]
                        )
                        nop.engine = inst.engine
                        nop.sync_info = mybir.SyncInfo(on_wait=[w], on_update=[])
                        nc.register_instruction(nop)
                        nops.append(nop)
                    inst.sync_info = mybir.SyncInfo(
                        on_wait=[waits[-1]], on_update=list(si.on_update)
                    )
                    insts[i:i] = nops
                    i += len(nops)
                i += 1
    return


def _get_nc() -> bass.Bass:
    global _NC_CACHE
    if _NC_CACHE is None:
        _NC_CACHE = _build()
    return _NC_CACHE


def _make_in_maps(x, Wp, bp, Wv):
    x = np.ascontiguousarray(np.asarray(x, dtype=np.float32))
    Wp = np.asarray(Wp, dtype=np.float32)
    bp = np.asarray(bp, dtype=np.float32)
    Wv = np.asarray(Wv, dtype=np.float32)

    # fold the tiny weights (O(D^2) host prep)
    p = np.arange(S, dtype=np.float32)
    pos = p @ Wp.T + bp                       # (S,)
    wv = Wv.sum(axis=0)                       # (D,) column sums
    bias8 = (pos * wv.sum()).astype(np.float32)
    bias_rpp = np.tile(bias8, RPP // S)       # (RPP,) pattern per in-partition row
    wb_row = np.concatenate([wv, bias_rpp])   # (D + RPP,)
    wb = np.ascontiguousarray(np.broadcast_to(wb_row, (P, D + RPP)), dtype=np.float32)

    xf = x.reshape(B * S * D)
    in_maps = []
    for i in range(N_CORES):
        shard = xf[i * ROWS * D : (i + 1) * ROWS * D].reshape(P, FREE)
        in_maps.append({"x": shard, "wb": wb})
    return in_maps


def _run(x, Wp, bp, Wv, trace=False, **spmd_kwargs):
    nc = _get_nc()
    in_maps = _make_in_maps(x, Wp, bp, Wv)
    res = run_bass_kernel_spmd(
        nc, in_maps, list(range(N_CORES)), trace=trace, **spmd_kwargs
    )
    parts = [
        np.asarray(res.results[i]["out"]).reshape(BPC, S, D) for i in range(N_CORES)
    ]
    return np.concatenate(parts, axis=0), res


def kernel(x, Wp, bp, Wv, Wk, Wq) -> np.ndarray:
    out, _ = _run(x, Wp, bp, Wv)
    return out


# revision 16
# speedup vs baseline: 1.0304x; 1.0304x over previous
"""Trainium2 Bass kernel for nn_Attention_Temp_1468878815458.

Math: the reference computes
    pos   = arange(S) @ Wp.T + bp                       # (S,)
    embed = x.squeeze(1) + pos[:, None]                 # (B,S,D)
    v/k/q = embed @ {Wv,Wk,Wq}.T
    scores[b,x,y]  = (sum_q queries[b,q,x]) * (sum_k keys[b,k,y])
    attention      = softmax(scores, axis=1)            # over x
    out[b,v,y]     = sum_x attention[b,x,y] * sum_n values[b,v,n]

Since softmax normalizes over axis=1 and is then *summed* over axis=1,
sum_x attention[b,x,y] == 1 exactly.  Therefore
    out[b,s,y] = sum_n values[b,s,n]
               = (x[b,0,s,:] + pos[s]) . wv      for every y,
where wv[d] = sum_n Wv[n,d].  The kernel streams x once, computes the
per-row weighted sum with wv, adds the per-s bias pos[s]*sum(wv), and
broadcasts the scalar across the last dim.

Sharding: pure data parallel over batch, 1024 batches per core.  Each
core's shard is viewed as (128 partitions, 6144 f32): partition p holds
64 consecutive rows (8 batches x 8 seq) contiguously -> fully
contiguous DMA in AND out (24KB runs/partition).
"""

import numpy as np

import concourse.bass as bass
import concourse.mybir as mybir
from concourse.bass import broadcast_tensor_aps
from concourse.bass_utils import run_bass_kernel_spmd
from concourse.tile import TileContext

N_CORES = 8
B, S, D = 8192, 8, 96
BPC = B // N_CORES          # 1024 batches per core
ROWS = BPC * S              # 8192 rows of length D per core
P = 128                     # SBUF partitions
FREE = ROWS * D // P        # 6144 f32 per partition
RPP = ROWS // P             # 64 rows per partition
NCH = 8                     # pipeline chunks
CHR = RPP // NCH            # rows per partition per chunk
CHF = CHR * D               # f32 per partition per chunk

_NC_CACHE = None


def _build() -> bass.Bass:
    # seq codegen lowers multi-wait sync (e.g. the kernel-tail drain) to
    # sequencer commands; this walrus build allows only 1 wait per inst
    nc = bass.Bass(use_seq_codegen=True)
    x = nc.declare_dram_parameter("x", [P, FREE], mybir.dt.float32, isOutput=False)
    # combined constants: [:, :D] = wv replicated, [:, D:D+RPP] = per-row bias
    wb = nc.declare_dram_parameter("wb", [P, D + RPP], mybir.dt.float32, isOutput=False)
    out = nc.declare_dram_parameter("out", [P, FREE], mybir.dt.float32, isOutput=True)

    with TileContext(nc) as tc:
        with (
            tc.tile_pool(name="const", bufs=1) as cpool,
            # one buffer per chunk: no slot reuse -> no WAR waits
            tc.tile_pool(name="xp", bufs=NCH) as xpool,
            tc.tile_pool(name="pp", bufs=4) as ppool,
            tc.tile_pool(name="op", bufs=NCH) as opool,
            tc.tile_pool(name="rp", bufs=NCH) as rpool,
        ):
            wb_sb = cpool.tile([P, D + RPP], mybir.dt.float32)
            # SWDGE: keeps the 8 HWDGE sem lanes free for the x/out streams
            nc.gpsimd.dma_start(out=wb_sb[:], in_=wb[:])
            wv_sb = wb_sb[:, :D]
            bias_sb = wb_sb[:, D : D + RPP]
            # priming op: absorbs the wb DMA wait so per-chunk TTs carry
            # only their own x-chunk DMA wait (HW limit: 1 wait per TT)
            prime = rpool.tile([P, 1], mybir.dt.float32, tag="prime")
            nc.vector.reduce_sum(
                out=prime[:], in_=wb_sb[:], axis=mybir.AxisListType.X
            )

            for c in range(NCH):
                xt = xpool.tile([P, CHF], mybir.dt.float32, tag="xt")
                nc.sync.dma_start(out=xt[:], in_=x[:, c * CHF : (c + 1) * CHF])

                x3 = xt[:].rearrange("p (r d) -> p r d", d=D)
                wv3 = wv_sb.rearrange("p (r d) -> p r d", r=1)
                _, wv3b = broadcast_tensor_aps(x3, wv3)
                pt = ppool.tile([P, CHF], mybir.dt.float32, tag="pt")
                p3 = pt[:].rearrange("p (r d) -> p r d", d=D)
                nc.vector.tensor_tensor(
                    out=p3, in0=x3, in1=wv3b, op=mybir.AluOpType.mult
                )

                rd = rpool.tile([P, CHR], mybir.dt.float32, tag="rd")
                nc.vector.reduce_sum(out=rd[:], in_=p3, axis=mybir.AxisListType.X)
                nc.vector.tensor_add(
                    out=rd[:], in0=rd[:], in1=bias_sb[:, c * CHR : (c + 1) * CHR]
                )

                ot = opool.tile([P, CHF], mybir.dt.float32, tag="ot")
                ot3 = ot[:].rearrange("p (r d) -> p r d", d=D)
                rd3 = rd[:].rearrange("p (r d) -> p r d", d=1)
                _, rd3b = broadcast_tensor_aps(ot3, rd3)
                nc.scalar.copy(out=ot3, in_=rd3b)

                # ACT HWDGE ring: decouples the out stream from the in
                # stream's SP-ring FIFO
                nc.scalar.dma_start(out=out[:, c * CHF : (c + 1) * CHF], in_=ot[:])
    _split_multi_waits(nc)
    return nc


def _split_multi_waits(nc: bass.Bass) -> None:
    """Walrus (this build) allows only one sync wait per instruction.

    Tile's kernel-tail drain merges waits on every DMA lane + engine sem
    into one instruction; split the extras onto same-engine NOPs placed
    immediately before it.
    """
    for f in nc.m.functions:
        for bb in f.blocks:
            insts = bb.instructions
            i = 0
            while i < len(insts):
                inst = insts[i]
                si = inst.sync_info
                if si is not None and si.on_wait and len(si.on_wait) > 1:
                    waits = list(si.on_wait)
                    nops = []
                    for j, w in enumerate(waits[:-1]):
                        nop = mybir.InstNoOp(
                            name=f"{inst.name}-wsplit{j}", ins=[], outs=[]
                        )
                        nop.engine = inst.engine
                        nop.sync_info = mybir.SyncInfo(on_wait=[w], on_update=[])
                        nc.register_instruction(nop)
                        nops.append(nop)
                    inst.sync_info = mybir.SyncInfo(
                        on_wait=[waits[-1]], on_update=list(si.on_update)
                    )
                    insts[i:i] = nops
                    i += len(nops)
                i += 1
    return


def _get_nc() -> bass.Bass:
    global _NC_CACHE
    if _NC_CACHE is None:
        _NC_CACHE = _build()
    return _NC_CACHE


def _make_in_maps(x, Wp, bp, Wv):
    x = np.ascontiguousarray(np.asarray(x, dtype=np.float32))
    Wp = np.asarray(Wp, dtype=np.float32)
    bp = np.asarray(bp, dtype=np.float32)
    Wv = np.asarray(Wv, dtype=np.float32)

    # fold the tiny weights (O(D^2) host prep)
    p = np.arange(S, dtype=np.float32)
    pos = p @ Wp.T + bp                       # (S,)
    wv = Wv.sum(axis=0)                       # (D,) column sums
    bias8 = (pos * wv.sum()).astype(np.float32)
    bias_rpp = np.tile(bias8, RPP // S)       # (RPP,) pattern per in-partition row
    wb_row = np.concatenate([wv, bias_rpp])   # (D + RPP,)
    wb = np.ascontiguousarray(np.broadcast_to(wb_row, (P, D + RPP)), dtype=np.float32)

    xf = x.reshape(B * S * D)
    in_maps = []
    for i in range(N_CORES):
        shard = xf[i * ROWS * D : (i + 1) * ROWS * D].reshape(P, FREE)
        in_maps.append({"x": shard, "wb": wb})
    return in_maps


def _run(x, Wp, bp, Wv, trace=False, **spmd_kwargs):
    nc = _get_nc()
    in_maps = _make_in_maps(x, Wp, bp, Wv)
    res = run_bass_kernel_spmd(
        nc, in_maps, list(range(N_CORES)), trace=trace, **spmd_kwargs
    )
    parts = [
        np.asarray(res.results[i]["out"]).reshape(BPC, S, D) for i in range(N_CORES)
    ]
    return np.concatenate(parts, axis=0), res


def kernel(x, Wp, bp, Wv, Wk, Wq) -> np.ndarray:
    out, _ = _run(x, Wp, bp, Wv)
    return out
